# revision 67
# baseline (speedup 1.0000x reference)
"""Trainium2 Bass kernel for an AttentionBlock (GroupNorm + single-layer MHA + proj residual).

Reference computation (per batch b):
    xn = GroupNorm(x[b])                        # 8 groups over C=256, HW spatial
    qkv = w_qkv @ xn                            # per-pixel 1x1 conv
    per head h (4 heads, d=64):
        scores = q_h^T k_h * d^-0.5             # [HW, HW]
        attn = softmax(scores, axis=keys)
        out_h = v_h @ attn^T                    # [d, HW]
    y = xn + w_proj @ concat(out_h) + b_proj

Sharding: 8 cores = (batch b in {0,1}) x (head h in {0..3}).  Each core runs
GroupNorm + its head's attention for all spatial positions, computes the
per-head projection partial for all positions, and a 4-way ReduceScatter
(within the batch group) sums the partials and hands each core its owned
quarter of the spatial axis.  The residual (xn + b_proj) for the owned
quarter is added post-collective.  Host work is pure unshard: concatenate
the 8 [256, HW/4] outputs.

Key kernel-level layout choice: scores are computed TRANSPOSED
(keys j on partitions, queries i on the free axis) so that
 - the PV contraction (over j) needs no transposes at all, and
 - the softmax denominator comes free as a 65th "ones" column of V
   (row 64 of the PV accumulator).
Softmax skips max-subtraction: scores are ~N(0,1) here so exp is safe in
fp32 (matches the reference up to fp rounding since the max subtraction
cancels exactly).  Scores are computed in the log2 domain (q pre-scaled by
d^-0.5*log2 e host-side; exp computed as 2^t) and the attention matmuls
run in bf16 with fp32 PSUM accumulation — fp32 matmuls on TRN2 cost 2x
(LOW_HIGH two-pass mode).

Measured on 8 axon TRN2 NeuronCores: ~237-255 us HW exec, rel err ~4e-4.
"""

import numpy as np

C = 256
NH = 4
D = 64
G = 8
EPS = 1e-5
B = 2
NCORES = 8
PDIM = 128  # partitions


def build_nc(HW: int):
    import concourse.bass as bass
    import concourse.mybir as mybir
    import concourse.tile as tile
    from concourse import bacc

    f32 = mybir.dt.float32
    bf16 = mybir.dt.bfloat16
    CW = min(512, HW)          # i-chunk width (matmul moving-operand max)
    NIC = HW // CW             # number of i-chunks
    NJT = HW // PDIM           # number of key tiles (128 keys each)
    OWN = HW // 4              # spatial columns owned by each core at output
    HALF = OWN // 2            # per-ReduceScatter-call window inside each owned slice
    assert NJT % 2 == 0 and OWN % 2 == 0

    nc = bacc.Bacc(
        "TRN2", target_bir_lowering=False, debug=False, num_devices=NCORES
    )

    xb = nc.declare_dram_parameter("xb", [C, HW], f32, isOutput=False)
    x_own = nc.declare_dram_parameter("x_own", [C, OWN], f32, isOutput=False)
    wqT = nc.declare_dram_parameter("wqT", [C, 2 * D], bf16, isOutput=False)
    wkT = nc.declare_dram_parameter("wkT", [C, 2 * D], bf16, isOutput=False)
    wvT = nc.declare_dram_parameter("wvT", [C, D], bf16, isOutput=False)
    wpTh = nc.declare_dram_parameter("wpTh", [D, C], bf16, isOutput=False)
    gamma = nc.declare_dram_parameter("gamma", [C], f32, isOutput=False)
    beta = nc.declare_dram_parameter("beta", [C], f32, isOutput=False)
    bproj = nc.declare_dram_parameter("bproj", [C], f32, isOutput=False)
    indf = nc.declare_dram_parameter("indf", [2, PDIM, G], f32, isOutput=False)
    indb = nc.declare_dram_parameter("indb", [2, G, PDIM], f32, isOutput=False)
    y = nc.declare_dram_parameter("y", [C, OWN], f32, isOutput=True)

    groups = [[0, 1, 2, 3], [4, 5, 6, 7]]
    Exp = mybir.ActivationFunctionType.Exp
    Sqrt = mybir.ActivationFunctionType.Sqrt
    MUL = mybir.AluOpType.mult
    ADD = mybir.AluOpType.add

    BNW = min(512, HW)         # bn_stats max free dim
    NBN = HW // BNW

    # i-chunk processing order: chunks feeding ReduceScatter call 0 first so
    # RS#0 overlaps with the remaining chunks' compute.
    def chunk_calls(c):
        # which RS calls (0/1) does chunk c's column range contribute to
        lo, hi = c * CW, (c + 1) * CW
        calls = set()
        for r in range(4):
            for k in range(2):
                wlo = r * OWN + k * HALF
                whi = wlo + HALF
                if max(lo, wlo) < min(hi, whi):
                    calls.add(k)
        return calls

    order = sorted(range(NIC), key=lambda c: (max(chunk_calls(c)), c))

    with tile.TileContext(nc) as tc:
        with (
            tc.tile_pool(name="consts", bufs=1) as consts,
            tc.tile_pool(name="xpool", bufs=1) as xpool,
            tc.tile_pool(name="xnpool", bufs=1) as xnpool,
            tc.tile_pool(name="gn_sm", bufs=2) as gn_sm,
            tc.tile_pool(name="qkpool", bufs=1) as qkpool,
            tc.tile_pool(name="espool", bufs=4) as espool,
            tc.tile_pool(name="mlsm", bufs=2) as mlsm,
            tc.tile_pool(name="ypool", bufs=2) as ypool,
            tc.tile_pool(name="dram", bufs=1, space="DRAM") as dram,
        ):
            # ---------------- x load first (biggest transfer, gates GN) ----------------
            x_sb = []
            xo_sb = []
            for t in range(2):
                xt = xpool.tile([PDIM, HW], f32, tag=f"x{t}", name=f"x{t}")
                for c in range(NIC):
                    nc.sync.dma_start(
                        out=xt[:, bass.ts(c, CW)],
                        in_=xb[bass.ts(t, PDIM), bass.ts(c, CW)],
                    )
                x_sb.append(xt)
                xo = xpool.tile([PDIM, OWN], f32, tag=f"xo{t}", name=f"xo{t}")
                nc.sync.dma_start(out=xo, in_=x_own[bass.ts(t, PDIM), :])
                xo_sb.append(xo)

            # ---------------- constants / small loads ----------------
            eps_t = consts.tile([PDIM, 1], f32)
            nc.vector.memset(eps_t, EPS)
            ones64 = consts.tile([1, D], bf16)
            nc.vector.memset(ones64, 1.0)


            indf_sb = []
            indb_sb = []
            gm_sb = []
            bt_sb = []
            bp_sb = []
            for t in range(2):
                it_ = consts.tile([PDIM, G], f32, tag=f"indf{t}")
                nc.sync.dma_start(out=it_, in_=indf[t])
                indf_sb.append(it_)
                ib_ = consts.tile([G, PDIM], f32, tag=f"indb{t}")
                nc.sync.dma_start(out=ib_, in_=indb[t])
                indb_sb.append(ib_)
                g_ = consts.tile([PDIM, 1], f32, tag=f"gm{t}")
                nc.sync.dma_start(out=g_, in_=gamma[bass.ts(t, PDIM)].rearrange("(p o) -> p o", o=1))
                gm_sb.append(g_)
                b_ = consts.tile([PDIM, 1], f32, tag=f"bt{t}")
                nc.sync.dma_start(out=b_, in_=beta[bass.ts(t, PDIM)].rearrange("(p o) -> p o", o=1))
                bt_sb.append(b_)
                bp_ = consts.tile([PDIM, 1], f32, tag=f"bp{t}")
                nc.sync.dma_start(out=bp_, in_=bproj[bass.ts(t, PDIM)].rearrange("(p o) -> p o", o=1))
                bp_sb.append(bp_)

            wq_sb = []
            wk_sb = []
            wv_sb = []
            for t in range(2):
                for (w_sb, src, w_, tag) in (
                    (wq_sb, wqT, 2 * D, "wq"),
                    (wk_sb, wkT, 2 * D, "wk"),
                    (wv_sb, wvT, D, "wv"),
                ):
                    wt = consts.tile([PDIM, w_], bf16, tag=f"{tag}{t}")
                    nc.sync.dma_start(out=wt, in_=src[bass.ts(t, PDIM), :])
                    w_sb.append(wt)
            wp_sb = consts.tile([D, C], bf16)
            nc.sync.dma_start(out=wp_sb, in_=wpTh[:, :])

            from contextlib import ExitStack

            ps_stack = ExitStack()
            gn_ps = ps_stack.enter_context(tc.tile_pool(name="gn_ps", bufs=1, space="PSUM"))
            qk_ps = ps_stack.enter_context(tc.tile_pool(name="qk_ps", bufs=3, space="PSUM"))

            gst_ps = gn_ps.tile([G, 2], f32, tag="gst")
            for t in range(2):
                stats = gn_sm.tile([PDIM, NBN, 6], f32, tag="bnst")
                for s in range(NBN):
                    nc.vector.bn_stats(out=stats[:, s, :], in_=x_sb[t][:, bass.ts(s, BNW)])
                mv = gn_sm.tile([PDIM, 2], f32, tag="mv")
                nc.vector.bn_aggr(out=mv, in_=stats)
                st2 = gn_sm.tile([PDIM, 2], f32, tag="st2")
                nc.vector.tensor_copy(st2[:, 0:1], mv[:, 0:1])
                sq = gn_sm.tile([PDIM, 1], f32, tag="sq")
                nc.vector.tensor_mul(sq, mv[:, 0:1], mv[:, 0:1])
                nc.vector.tensor_add(st2[:, 1:2], mv[:, 1:2], sq)
                nc.tensor.matmul(
                    out=gst_ps, lhsT=indf_sb[t], rhs=st2, start=(t == 0), stop=(t == 1)
                )

            gst = gn_sm.tile([G, 2], f32, tag="gst_sb")
            nc.vector.tensor_copy(gst, gst_ps)
            mu2 = gn_sm.tile([G, 1], f32, tag="mu2")
            nc.vector.tensor_mul(mu2, gst[:, 0:1], gst[:, 0:1])
            var = gn_sm.tile([G, 1], f32, tag="var")
            nc.vector.tensor_sub(var, gst[:, 1:2], mu2)
            sd = gn_sm.tile([G, 1], f32, tag="sd")
            nc.scalar.activation(out=sd, in_=var, func=Sqrt, bias=eps_t[0:G, :], scale=1.0)
            rstd = gn_sm.tile([G, 1], f32, tag="rstd")
            nc.vector.reciprocal(out=rstd, in_=sd)
            gmr = gn_sm.tile([G, 2], f32, tag="gmr")
            nc.vector.tensor_copy(gmr[:, 0:1], gst[:, 0:1])
            nc.vector.tensor_copy(gmr[:, 1:2], rstd)

            # per-channel affine params + normalized x + residual slice
            xn_sb = []
            resid_sb = []
            for t in range(2):
                gb_ps = gn_ps.tile([PDIM, 2], f32, tag="gb")
                nc.tensor.matmul(out=gb_ps, lhsT=indb_sb[t], rhs=gmr, start=True, stop=True)
                gb = gn_sm.tile([PDIM, 2], f32, tag="gb_sb")
                nc.vector.tensor_copy(gb, gb_ps)
                A_t = gn_sm.tile([PDIM, 1], f32, tag=f"A{t}")
                nc.vector.tensor_mul(A_t, gb[:, 1:2], gm_sb[t])
                tmp = gn_sm.tile([PDIM, 1], f32, tag="tmp")
                nc.vector.tensor_mul(tmp, gb[:, 0:1], A_t)
                B_t = gn_sm.tile([PDIM, 1], f32, tag=f"B{t}")
                nc.vector.tensor_sub(B_t, bt_sb[t], tmp)
                B2_t = gn_sm.tile([PDIM, 1], f32, tag=f"B2{t}")
                nc.vector.tensor_add(B2_t, B_t, bp_sb[t])

                xn_t = xnpool.tile([PDIM, HW], bf16, tag=f"xn{t}")
                nc.vector.tensor_scalar(xn_t, x_sb[t], A_t, B_t, MUL, ADD)
                xn_sb.append(xn_t)
                rs_t = xnpool.tile([PDIM, OWN], f32, tag=f"res{t}")
                nc.vector.tensor_scalar(rs_t, xo_sb[t], A_t, B2_t, MUL, ADD)
                resid_sb.append(rs_t)

            # ---------------- q, k, v_aug ----------------
            # q/k duplicated across both partition halves (weight columns are
            # pre-duplicated host-side) so scores pairs can row-pack the PE.
            q_sb = qkpool.tile([PDIM, HW], bf16, tag="q")
            k_sb = qkpool.tile([PDIM, HW], bf16, tag="k")
            for (dst, w_sb) in ((q_sb, wq_sb), (k_sb, wk_sb)):
                for c in range(NIC):
                    ps = qk_ps.tile([PDIM, CW], f32, tag="qk")
                    for t in range(2):
                        nc.tensor.matmul(
                            out=ps,
                            lhsT=w_sb[t],
                            rhs=xn_sb[t][:, bass.ts(c, CW)],
                            start=(t == 0),
                            stop=(t == 1),
                        )
                    # ScalarE is idle during this phase; give it the psum
                    # drains so the Vector engine keeps up with GN/stats work
                    nc.scalar.copy(dst[:, bass.ts(c, CW)], ps)

            v_aug = qkpool.tile([PDIM, NJT, D + 1], bf16, tag="vaug")
            nc.vector.memset(v_aug[:, :, D : D + 1], 1.0)
            for jt in range(NJT):
                ps = qk_ps.tile([PDIM, D], f32, tag="vt")
                for t in range(2):
                    nc.tensor.matmul(
                        out=ps,
                        lhsT=xn_sb[t][:, bass.ts(jt, PDIM)],
                        rhs=wv_sb[t],
                        start=(t == 0),
                        stop=(t == 1),
                    )
                nc.vector.tensor_copy(v_aug[:, jt, 0:D], ps)

            # ---------------- main attention loop ----------------
            ps_stack.close()  # release GN/QKV PSUM banks
            ps_stack2 = ExitStack()
            sc_ps = ps_stack2.enter_context(tc.tile_pool(name="sc_ps", bufs=2, space="PSUM"))
            pv_ps_pool = ps_stack2.enter_context(tc.tile_pool(name="pv_ps", bufs=1, space="PSUM"))
            pj_ps_pool = ps_stack2.enter_context(tc.tile_pool(name="pj_ps", bufs=1, space="PSUM"))

            rs_in = [
                dram.tile([4, C, HALF], bf16, name=f"rsin{k}", tag=f"rsin{k}")
                for k in range(2)
            ]
            rs_out = [
                dram.tile([C, HALF], bf16, name=f"rsout{k}", tag=f"rsout{k}")
                for k in range(2)
            ]
            rs_done = [False, False]

            def emit_rs(k):
                nc.gpsimd.collective_compute(
                    "ReduceScatter",
                    mybir.AluOpType.add,
                    replica_groups=groups,
                    ins=[rs_in[k].opt()],
                    outs=[rs_out[k].opt()],
                )
                rs_done[k] = True

            done_chunks = set()
            GSZ = 3  # j-tiles per exp group: FD=1536 amortizes ACT overhead
            for c in order:
                cslice = bass.ts(c, CW)
                pv = pv_ps_pool.tile([D + 1, CW], f32, tag="pv")
                ps_tiles = {}
                es_tiles = {}
                written = {}
                for p in range(NJT // 2):
                    # two K=64 matmuls packed into disjoint PE row-groups
                    # (tile_position auto-derives from base_partition 0 / 64);
                    # each lands in its own slot of a 3-wide exp-group tile,
                    # so pairs may straddle group boundaries.
                    for s in range(2):
                        jt = 2 * p + s
                        g = jt // GSZ
                        if g not in ps_tiles:
                            size = min(GSZ, NJT - GSZ * g)
                            ps_tiles[g] = sc_ps.tile(
                                [PDIM, size * CW], f32, tag="sc", name=f"sc{c}_{g}"
                            )
                            es_tiles[g] = espool.tile(
                                [PDIM, size * CW], bf16, tag="es", name=f"es{c}_{g}"
                            )
                            written[g] = 0
                        nc.tensor.matmul(
                            out=ps_tiles[g][:, bass.ts(jt % GSZ, CW)],
                            lhsT=k_sb[s * D : (s + 1) * D, bass.ts(jt, PDIM)],
                            rhs=q_sb[s * D : (s + 1) * D, cslice],
                            start=True,
                            stop=True,
                        )
                        written[g] += 1
                    for g in sorted(ps_tiles):
                        size = min(GSZ, NJT - GSZ * g)
                        if written[g] < size:
                            continue
                        # scores are already in the log2 domain (q pre-scaled
                        # by d^-0.5*log2(e) host-side): es = 2^t = exp(ln2*t)
                        nc.scalar.activation(
                            out=es_tiles[g],
                            in_=ps_tiles[g],
                            func=Exp,
                            scale=0.6931471805599453,
                        )
                        for sl in range(size):
                            jt = GSZ * g + sl
                            nc.tensor.matmul(
                                out=pv,
                                lhsT=v_aug[:, jt, :],
                                rhs=es_tiles[g][:, bass.ts(sl, CW)],
                                start=(jt == 0),
                                stop=(jt == NJT - 1),
                            )
                        del ps_tiles[g], es_tiles[g], written[g]

                # normalize: out_norm = pv[0:64] * (1/denom) broadcast over
                # partitions via a K=1 ones-matmul (psum slot shared with proj)
                den = mlsm.tile([1, CW], f32, tag="den")
                nc.vector.tensor_copy(den, pv[D : D + 1, :])
                rden = mlsm.tile([1, CW], f32, tag="rden")
                nc.vector.reciprocal_approx_fast(out=rden, in_=den)
                rdenb = mlsm.tile([1, CW], bf16, tag="rdenb")
                nc.vector.tensor_copy(rdenb, rden)
                bc = pj_ps_pool.tile([D, CW], f32, name="bc", tag="pj")
                nc.tensor.matmul(out=bc, lhsT=ones64, rhs=rdenb, start=True, stop=True)
                rdb = mlsm.tile([D, CW], f32, tag="rdb")
                nc.vector.tensor_copy(rdb, bc)
                onorm = mlsm.tile([D, CW], bf16, tag="onorm")
                nc.vector.tensor_mul(onorm, pv[0:D, :], rdb)

                # projection partial for this i-chunk: [256, CW]
                # (one PSUM bank, serial over the two output-channel halves)
                yp = []
                for co in range(2):
                    pj = pj_ps_pool.tile([PDIM, CW], f32, name=f"pj{co}", tag="pj")
                    nc.tensor.matmul(
                        out=pj,
                        lhsT=wp_sb[:, bass.ts(co, PDIM)],
                        rhs=onorm,
                        start=True,
                        stop=True,
                    )
                    yt = ypool.tile([PDIM, CW], bf16, tag="yp")
                    nc.vector.tensor_copy(yt, pj)
                    yp.append(yt)

                # scatter this chunk's columns into the RS input windows
                lo, hi = c * CW, (c + 1) * CW
                for r in range(4):
                    for k in range(2):
                        wlo = r * OWN + k * HALF
                        whi = wlo + HALF
                        a, b_ = max(lo, wlo), min(hi, whi)
                        if a >= b_:
                            continue
                        for co in range(2):
                            last_loop_inst = nc.sync.dma_start(
                                out=rs_in[k][r, bass.ts(co, PDIM), a - wlo : b_ - wlo],
                                in_=yp[co][:, a - lo : b_ - lo],
                            )

                done_chunks.add(c)

            # Emit collectives only after ALL compute is emitted: instructions
            # emitted after a collective get event-chained behind its (long)
            # occupancy of the gpsimd queue, stalling the whole machine while
            # it runs.  Emitted last, each RS still fires as soon as its
            # rs_in writes complete (semaphore deps, not emission order).
            for k in range(2):
                if not rs_done[k]:
                    emit_rs(k)

            # ---------------- post-collective: add residual, write y ----------------
            # Ordering-only deps keep these AFTER all loop compute in each
            # engine's (strictly in-order) queue — otherwise the scheduler can
            # interleave them mid-loop and the vector/sync engines block on the
            # collective, stalling the whole machine while it runs.
            from concourse.tile import add_dep_helper

            for k in range(2):
                for t in range(2):
                    ro = ypool.tile([PDIM, HALF], bf16, tag="ro")
                    i1 = nc.sync.dma_start(out=ro, in_=rs_out[k][bass.ts(t, PDIM), :])
                    yf = ypool.tile([PDIM, HALF], f32, tag="yf")
                    i2 = nc.vector.tensor_add(yf, ro, resid_sb[t][:, bass.ts(k, HALF)])
                    i3 = nc.sync.dma_start(
                        out=y[bass.ts(t, PDIM), bass.ts(k, HALF)], in_=yf
                    )
                    for ii in (i1, i2, i3):
                        add_dep_helper(
                            ii.ins,
                            last_loop_inst.ins,
                            sync=False,
                            reason="keep post-collective epilogue after loop compute",
                        )

            ps_stack2.close()

    nc.compile()
    return nc


def make_in_maps(x, gn_gamma, gn_beta, w_qkv, w_proj, b_proj, HW):
    """Per-core input dicts. Core c = (b = c//4, h = c%4)."""
    import ml_dtypes

    bf16 = ml_dtypes.bfloat16
    OWN = HW // 4
    x2 = np.ascontiguousarray(x.reshape(B, C, HW).astype(np.float32))
    w_qkv = np.asarray(w_qkv, dtype=np.float32)
    w_proj = np.asarray(w_proj, dtype=np.float32)
    indf = np.zeros((2, PDIM, G), dtype=np.float32)
    indb = np.zeros((2, G, PDIM), dtype=np.float32)
    gsz = C // G  # 32 channels per group
    for t in range(2):
        for p in range(PDIM):
            g = (t * PDIM + p) // gsz
            indf[t, p, g] = 1.0 / gsz
            indb[t, g, p] = 1.0
    in_maps = []
    for c in range(NCORES):
        b, h = c // 4, c % 4
        in_maps.append(
            {
                "xb": x2[b],
                "x_own": np.ascontiguousarray(x2[b][:, h * OWN : (h + 1) * OWN]),
                "wqT": np.ascontiguousarray(
                    np.tile(w_qkv[0 * C + h * D : 0 * C + (h + 1) * D, :].T, (1, 2))
                    * (D ** -0.5 * np.log2(np.e))
                ).astype(bf16),
                "wkT": np.ascontiguousarray(
                    np.tile(w_qkv[1 * C + h * D : 1 * C + (h + 1) * D, :].T, (1, 2))
                ).astype(bf16),
                "wvT": np.ascontiguousarray(w_qkv[2 * C + h * D : 2 * C + (h + 1) * D, :].T).astype(bf16),
                "wpTh": np.ascontiguousarray(w_proj[:, h * D : (h + 1) * D].T).astype(bf16),
                "gamma": np.asarray(gn_gamma, dtype=np.float32),
                "beta": np.asarray(gn_beta, dtype=np.float32),
                "bproj": np.asarray(b_proj, dtype=np.float32),
                "indf": indf,
                "indb": indb,
            }
        )
    return in_maps


def assemble_output(results, HW, Himg, Wimg):
    OWN = HW // 4
    y = np.empty((B, C, HW), dtype=np.float32)
    for c in range(NCORES):
        b, h = c // 4, c % 4
        y[b][:, h * OWN : (h + 1) * OWN] = results[c]["y"]
    return y.reshape(B, C, Himg, Wimg)


_NC_CACHE = {}


def kernel(x, gn_gamma, gn_beta, w_qkv, w_proj, b_proj):
    from concourse.bass_utils import run_bass_kernel_spmd

    Himg, Wimg = x.shape[2], x.shape[3]
    HW = Himg * Wimg
    if HW not in _NC_CACHE:
        _NC_CACHE[HW] = build_nc(HW)
    nc = _NC_CACHE[HW]
    in_maps = make_in_maps(x, gn_gamma, gn_beta, w_qkv, w_proj, b_proj, HW)
    res = run_bass_kernel_spmd(nc, in_maps, list(range(NCORES)))
    return assemble_output(res.results, HW, Himg, Wimg)


# revision 69
# speedup vs baseline: 1.0409x; 1.0409x over previous
"""Trainium2 Bass kernel for an AttentionBlock (GroupNorm + single-layer MHA + proj residual).

Reference computation (per batch b):
    xn = GroupNorm(x[b])                        # 8 groups over C=256, HW spatial
    qkv = w_qkv @ xn                            # per-pixel 1x1 conv
    per head h (4 heads, d=64):
        scores = q_h^T k_h * d^-0.5             # [HW, HW]
        attn = softmax(scores, axis=keys)
        out_h = v_h @ attn^T                    # [d, HW]
    y = xn + w_proj @ concat(out_h) + b_proj

Sharding: 8 cores = (batch b in {0,1}) x (head h in {0..3}).  Each core runs
GroupNorm + its head's attention for all spatial positions, computes the
per-head projection partial for all positions, and a 4-way ReduceScatter
(within the batch group) sums the partials and hands each core its owned
quarter of the spatial axis.  The residual (xn + b_proj) for the owned
quarter is added post-collective.  Host work is pure unshard: concatenate
the 8 [256, HW/4] outputs.

Key kernel-level layout choice: scores are computed TRANSPOSED
(keys j on partitions, queries i on the free axis) so that
 - the PV contraction (over j) needs no transposes at all, and
 - the softmax denominator comes free as a 65th "ones" column of V
   (row 64 of the PV accumulator).
Softmax skips max-subtraction: scores are ~N(0,1) here so exp is safe in
fp32 (matches the reference up to fp rounding since the max subtraction
cancels exactly).  Scores are computed in the log2 domain (q pre-scaled by
d^-0.5*log2 e host-side; exp computed as 2^t) and the attention matmuls
run in bf16 with fp32 PSUM accumulation — fp32 matmuls on TRN2 cost 2x
(LOW_HIGH two-pass mode).

Measured on 8 axon TRN2 NeuronCores: ~237-255 us HW exec, rel err ~4e-4.
"""

import numpy as np

C = 256
NH = 4
D = 64
G = 8
EPS = 1e-5
B = 2
NCORES = 8
PDIM = 128  # partitions


def build_nc(HW: int):
    import concourse.bass as bass
    import concourse.mybir as mybir
    import concourse.tile as tile
    from concourse import bacc

    f32 = mybir.dt.float32
    bf16 = mybir.dt.bfloat16
    CW = min(512, HW)          # i-chunk width (matmul moving-operand max)
    NIC = HW // CW             # number of i-chunks
    NJT = HW // PDIM           # number of key tiles (128 keys each)
    OWN = HW // 4              # spatial columns owned by each core at output
    HALF = OWN // 2            # per-ReduceScatter-call window inside each owned slice
    assert NJT % 2 == 0 and OWN % 2 == 0

    nc = bacc.Bacc(
        "TRN2", target_bir_lowering=False, debug=False, num_devices=NCORES
    )

    xb = nc.declare_dram_parameter("xb", [C, HW], f32, isOutput=False)
    x_own = nc.declare_dram_parameter("x_own", [C, OWN], f32, isOutput=False)
    wqT = nc.declare_dram_parameter("wqT", [C, 2 * D], bf16, isOutput=False)
    wkT = nc.declare_dram_parameter("wkT", [C, 2 * D], bf16, isOutput=False)
    wvT = nc.declare_dram_parameter("wvT", [C, D], bf16, isOutput=False)
    wpTh = nc.declare_dram_parameter("wpTh", [D, C], bf16, isOutput=False)
    gamma = nc.declare_dram_parameter("gamma", [C], f32, isOutput=False)
    beta = nc.declare_dram_parameter("beta", [C], f32, isOutput=False)
    bproj = nc.declare_dram_parameter("bproj", [C], f32, isOutput=False)
    indf = nc.declare_dram_parameter("indf", [2, PDIM, G], f32, isOutput=False)
    indb = nc.declare_dram_parameter("indb", [2, G, PDIM], f32, isOutput=False)
    y = nc.declare_dram_parameter("y", [C, OWN], f32, isOutput=True)

    groups = [[0, 1, 2, 3], [4, 5, 6, 7]]
    Exp = mybir.ActivationFunctionType.Exp
    Sqrt = mybir.ActivationFunctionType.Sqrt
    MUL = mybir.AluOpType.mult
    ADD = mybir.AluOpType.add

    BNW = min(512, HW)         # bn_stats max free dim
    NBN = HW // BNW

    # i-chunk processing order: chunks feeding ReduceScatter call 0 first so
    # RS#0 overlaps with the remaining chunks' compute.
    def chunk_calls(c):
        # which RS calls (0/1) does chunk c's column range contribute to
        lo, hi = c * CW, (c + 1) * CW
        calls = set()
        for r in range(4):
            for k in range(2):
                wlo = r * OWN + k * HALF
                whi = wlo + HALF
                if max(lo, wlo) < min(hi, whi):
                    calls.add(k)
        return calls

    order = sorted(range(NIC), key=lambda c: (max(chunk_calls(c)), c))

    with tile.TileContext(nc) as tc:
        with (
            tc.tile_pool(name="consts", bufs=1) as consts,
            tc.tile_pool(name="xpool", bufs=1) as xpool,
            tc.tile_pool(name="xnpool", bufs=1) as xnpool,
            tc.tile_pool(name="gn_sm", bufs=2) as gn_sm,
            tc.tile_pool(name="qkpool", bufs=1) as qkpool,
            tc.tile_pool(name="espool", bufs=4) as espool,
            tc.tile_pool(name="mlsm", bufs=2) as mlsm,
            tc.tile_pool(name="ypool", bufs=2) as ypool,
            tc.tile_pool(name="dram", bufs=1, space="DRAM") as dram,
        ):
            # ---------------- x load first (biggest transfer, gates GN) ----------------
            x_sb = []
            xo_sb = []
            for t in range(2):
                xt = xpool.tile([PDIM, HW], f32, tag=f"x{t}", name=f"x{t}")
                for c in range(NIC):
                    nc.sync.dma_start(
                        out=xt[:, bass.ts(c, CW)],
                        in_=xb[bass.ts(t, PDIM), bass.ts(c, CW)],
                    )
                x_sb.append(xt)
                xo = xpool.tile([PDIM, OWN], f32, tag=f"xo{t}", name=f"xo{t}")
                nc.sync.dma_start(out=xo, in_=x_own[bass.ts(t, PDIM), :])
                xo_sb.append(xo)

            # ---------------- constants / small loads ----------------
            eps_t = consts.tile([PDIM, 1], f32)
            nc.vector.memset(eps_t, EPS)
            ones128 = consts.tile([1, PDIM], bf16)
            nc.vector.memset(ones128, 1.0)


            indf_sb = []
            indb_sb = []
            gm_sb = []
            bt_sb = []
            bp_sb = []
            for t in range(2):
                it_ = consts.tile([PDIM, G], f32, tag=f"indf{t}")
                nc.sync.dma_start(out=it_, in_=indf[t])
                indf_sb.append(it_)
                ib_ = consts.tile([G, PDIM], f32, tag=f"indb{t}")
                nc.sync.dma_start(out=ib_, in_=indb[t])
                indb_sb.append(ib_)
                g_ = consts.tile([PDIM, 1], f32, tag=f"gm{t}")
                nc.sync.dma_start(out=g_, in_=gamma[bass.ts(t, PDIM)].rearrange("(p o) -> p o", o=1))
                gm_sb.append(g_)
                b_ = consts.tile([PDIM, 1], f32, tag=f"bt{t}")
                nc.sync.dma_start(out=b_, in_=beta[bass.ts(t, PDIM)].rearrange("(p o) -> p o", o=1))
                bt_sb.append(b_)
                bp_ = consts.tile([PDIM, 1], f32, tag=f"bp{t}")
                nc.sync.dma_start(out=bp_, in_=bproj[bass.ts(t, PDIM)].rearrange("(p o) -> p o", o=1))
                bp_sb.append(bp_)

            wq_sb = []
            wk_sb = []
            wv_sb = []
            for t in range(2):
                for (w_sb, src, w_, tag) in (
                    (wq_sb, wqT, 2 * D, "wq"),
                    (wk_sb, wkT, 2 * D, "wk"),
                    (wv_sb, wvT, D, "wv"),
                ):
                    wt = consts.tile([PDIM, w_], bf16, tag=f"{tag}{t}")
                    nc.sync.dma_start(out=wt, in_=src[bass.ts(t, PDIM), :])
                    w_sb.append(wt)
            wp_sb = consts.tile([D, C], bf16)
            nc.sync.dma_start(out=wp_sb, in_=wpTh[:, :])

            from contextlib import ExitStack

            ps_stack = ExitStack()
            gn_ps = ps_stack.enter_context(tc.tile_pool(name="gn_ps", bufs=1, space="PSUM"))
            qk_ps = ps_stack.enter_context(tc.tile_pool(name="qk_ps", bufs=3, space="PSUM"))

            gst_ps = gn_ps.tile([G, 2], f32, tag="gst")
            for t in range(2):
                stats = gn_sm.tile([PDIM, NBN, 6], f32, tag="bnst")
                for s in range(NBN):
                    nc.vector.bn_stats(out=stats[:, s, :], in_=x_sb[t][:, bass.ts(s, BNW)])
                mv = gn_sm.tile([PDIM, 2], f32, tag="mv")
                nc.vector.bn_aggr(out=mv, in_=stats)
                st2 = gn_sm.tile([PDIM, 2], f32, tag="st2")
                nc.vector.tensor_copy(st2[:, 0:1], mv[:, 0:1])
                sq = gn_sm.tile([PDIM, 1], f32, tag="sq")
                nc.vector.tensor_mul(sq, mv[:, 0:1], mv[:, 0:1])
                nc.vector.tensor_add(st2[:, 1:2], mv[:, 1:2], sq)
                nc.tensor.matmul(
                    out=gst_ps, lhsT=indf_sb[t], rhs=st2, start=(t == 0), stop=(t == 1)
                )

            gst = gn_sm.tile([G, 2], f32, tag="gst_sb")
            nc.vector.tensor_copy(gst, gst_ps)
            mu2 = gn_sm.tile([G, 1], f32, tag="mu2")
            nc.vector.tensor_mul(mu2, gst[:, 0:1], gst[:, 0:1])
            var = gn_sm.tile([G, 1], f32, tag="var")
            nc.vector.tensor_sub(var, gst[:, 1:2], mu2)
            sd = gn_sm.tile([G, 1], f32, tag="sd")
            nc.scalar.activation(out=sd, in_=var, func=Sqrt, bias=eps_t[0:G, :], scale=1.0)
            rstd = gn_sm.tile([G, 1], f32, tag="rstd")
            nc.vector.reciprocal(out=rstd, in_=sd)
            gmr = gn_sm.tile([G, 2], f32, tag="gmr")
            nc.vector.tensor_copy(gmr[:, 0:1], gst[:, 0:1])
            nc.vector.tensor_copy(gmr[:, 1:2], rstd)

            # per-channel affine params + normalized x + residual slice
            xn_sb = []
            resid_sb = []
            for t in range(2):
                gb_ps = gn_ps.tile([PDIM, 2], f32, tag="gb")
                nc.tensor.matmul(out=gb_ps, lhsT=indb_sb[t], rhs=gmr, start=True, stop=True)
                gb = gn_sm.tile([PDIM, 2], f32, tag="gb_sb")
                nc.vector.tensor_copy(gb, gb_ps)
                A_t = gn_sm.tile([PDIM, 1], f32, tag=f"A{t}")
                nc.vector.tensor_mul(A_t, gb[:, 1:2], gm_sb[t])
                tmp = gn_sm.tile([PDIM, 1], f32, tag="tmp")
                nc.vector.tensor_mul(tmp, gb[:, 0:1], A_t)
                B_t = gn_sm.tile([PDIM, 1], f32, tag=f"B{t}")
                nc.vector.tensor_sub(B_t, bt_sb[t], tmp)
                B2_t = gn_sm.tile([PDIM, 1], f32, tag=f"B2{t}")
                nc.vector.tensor_add(B2_t, B_t, bp_sb[t])

                xn_t = xnpool.tile([PDIM, HW], bf16, tag=f"xn{t}")
                nc.vector.tensor_scalar(xn_t, x_sb[t], A_t, B_t, MUL, ADD)
                xn_sb.append(xn_t)
                rs_t = xnpool.tile([PDIM, OWN], f32, tag=f"res{t}")
                nc.vector.tensor_scalar(rs_t, xo_sb[t], A_t, B2_t, MUL, ADD)
                resid_sb.append(rs_t)

            # ---------------- q, k, v_aug ----------------
            # q/k duplicated across both partition halves (weight columns are
            # pre-duplicated host-side) so scores pairs can row-pack the PE.
            q_sb = qkpool.tile([PDIM, HW], bf16, tag="q")
            k_sb = qkpool.tile([PDIM, HW], bf16, tag="k")
            for (dst, w_sb) in ((q_sb, wq_sb), (k_sb, wk_sb)):
                for c in range(NIC):
                    ps = qk_ps.tile([PDIM, CW], f32, tag="qk")
                    for t in range(2):
                        nc.tensor.matmul(
                            out=ps,
                            lhsT=w_sb[t],
                            rhs=xn_sb[t][:, bass.ts(c, CW)],
                            start=(t == 0),
                            stop=(t == 1),
                        )
                    # ScalarE is idle during this phase; give it the psum
                    # drains so the Vector engine keeps up with GN/stats work
                    nc.scalar.copy(dst[:, bass.ts(c, CW)], ps)

            v_aug = qkpool.tile([PDIM, NJT, D + 1], bf16, tag="vaug")
            nc.vector.memset(v_aug[:, :, D : D + 1], 1.0)
            for jt in range(NJT):
                ps = qk_ps.tile([PDIM, D], f32, tag="vt")
                for t in range(2):
                    nc.tensor.matmul(
                        out=ps,
                        lhsT=xn_sb[t][:, bass.ts(jt, PDIM)],
                        rhs=wv_sb[t],
                        start=(t == 0),
                        stop=(t == 1),
                    )
                nc.vector.tensor_copy(v_aug[:, jt, 0:D], ps)

            # ---------------- main attention loop ----------------
            ps_stack.close()  # release GN/QKV PSUM banks
            ps_stack2 = ExitStack()
            sc_ps = ps_stack2.enter_context(tc.tile_pool(name="sc_ps", bufs=2, space="PSUM"))
            pv_ps_pool = ps_stack2.enter_context(tc.tile_pool(name="pv_ps", bufs=1, space="PSUM"))
            pj_ps_pool = ps_stack2.enter_context(tc.tile_pool(name="pj_ps", bufs=1, space="PSUM"))

            rs_in = [
                dram.tile([4, C, HALF], bf16, name=f"rsin{k}", tag=f"rsin{k}")
                for k in range(2)
            ]
            rs_out = [
                dram.tile([C, HALF], bf16, name=f"rsout{k}", tag=f"rsout{k}")
                for k in range(2)
            ]
            rs_done = [False, False]

            def emit_rs(k):
                nc.gpsimd.collective_compute(
                    "ReduceScatter",
                    mybir.AluOpType.add,
                    replica_groups=groups,
                    ins=[rs_in[k].opt()],
                    outs=[rs_out[k].opt()],
                )
                rs_done[k] = True

            done_chunks = set()
            GSZ = 3  # j-tiles per exp group: FD=1536 amortizes ACT overhead
            for c in order:
                cslice = bass.ts(c, CW)
                pv = pv_ps_pool.tile([D + 1, CW], f32, tag="pv")
                ps_tiles = {}
                es_tiles = {}
                written = {}
                for p in range(NJT // 2):
                    # two K=64 matmuls packed into disjoint PE row-groups
                    # (tile_position auto-derives from base_partition 0 / 64);
                    # each lands in its own slot of a 3-wide exp-group tile,
                    # so pairs may straddle group boundaries.
                    for s in range(2):
                        jt = 2 * p + s
                        g = jt // GSZ
                        if g not in ps_tiles:
                            size = min(GSZ, NJT - GSZ * g)
                            ps_tiles[g] = sc_ps.tile(
                                [PDIM, size * CW], f32, tag="sc", name=f"sc{c}_{g}"
                            )
                            es_tiles[g] = espool.tile(
                                [PDIM, size * CW], bf16, tag="es", name=f"es{c}_{g}"
                            )
                            written[g] = 0
                        nc.tensor.matmul(
                            out=ps_tiles[g][:, bass.ts(jt % GSZ, CW)],
                            lhsT=k_sb[s * D : (s + 1) * D, bass.ts(jt, PDIM)],
                            rhs=q_sb[s * D : (s + 1) * D, cslice],
                            start=True,
                            stop=True,
                        )
                        written[g] += 1
                    for g in sorted(ps_tiles):
                        size = min(GSZ, NJT - GSZ * g)
                        if written[g] < size:
                            continue
                        # scores are already in the log2 domain (q pre-scaled
                        # by d^-0.5*log2(e) host-side): es = 2^t = exp(ln2*t)
                        nc.scalar.activation(
                            out=es_tiles[g],
                            in_=ps_tiles[g],
                            func=Exp,
                            scale=0.6931471805599453,
                        )
                        for sl in range(size):
                            jt = GSZ * g + sl
                            nc.tensor.matmul(
                                out=pv,
                                lhsT=v_aug[:, jt, :],
                                rhs=es_tiles[g][:, bass.ts(sl, CW)],
                                start=(jt == 0),
                                stop=(jt == NJT - 1),
                            )
                        del ps_tiles[g], es_tiles[g], written[g]

                # Free the PV accumulator fast (2 copies), then project the
                # UNNORMALIZED output and scale the projection result instead:
                # softmax normalization commutes with the linear projection.
                den = mlsm.tile([1, CW], f32, tag="den")
                nc.vector.tensor_copy(den, pv[D : D + 1, :])
                oraw = mlsm.tile([D, CW], bf16, tag="oraw")
                nc.vector.tensor_copy(oraw, pv[0:D, :])
                rden = mlsm.tile([1, CW], f32, tag="rden")
                nc.vector.reciprocal_approx_fast(out=rden, in_=den)
                rdenb = mlsm.tile([1, CW], bf16, tag="rdenb")
                nc.vector.tensor_copy(rdenb, rden)
                bc = pj_ps_pool.tile([PDIM, CW], f32, name="bc", tag="pj")
                nc.tensor.matmul(out=bc, lhsT=ones128, rhs=rdenb, start=True, stop=True)
                rdb = mlsm.tile([PDIM, CW], f32, tag="rdb")
                nc.vector.tensor_copy(rdb, bc)

                # projection partial for this i-chunk: [256, CW]
                # (one PSUM bank, serial over the two output-channel halves)
                yp = []
                for co in range(2):
                    pj = pj_ps_pool.tile([PDIM, CW], f32, name=f"pj{co}", tag="pj")
                    nc.tensor.matmul(
                        out=pj,
                        lhsT=wp_sb[:, bass.ts(co, PDIM)],
                        rhs=oraw,
                        start=True,
                        stop=True,
                    )
                    yt = ypool.tile([PDIM, CW], bf16, tag="yp")
                    nc.vector.tensor_mul(yt, pj, rdb)
                    yp.append(yt)

                # scatter this chunk's columns into the RS input windows
                lo, hi = c * CW, (c + 1) * CW
                for r in range(4):
                    for k in range(2):
                        wlo = r * OWN + k * HALF
                        whi = wlo + HALF
                        a, b_ = max(lo, wlo), min(hi, whi)
                        if a >= b_:
                            continue
                        for co in range(2):
                            last_loop_inst = nc.sync.dma_start(
                                out=rs_in[k][r, bass.ts(co, PDIM), a - wlo : b_ - wlo],
                                in_=yp[co][:, a - lo : b_ - lo],
                            )

                done_chunks.add(c)

            # Emit collectives only after ALL compute is emitted: instructions
            # emitted after a collective get event-chained behind its (long)
            # occupancy of the gpsimd queue, stalling the whole machine while
            # it runs.  Emitted last, each RS still fires as soon as its
            # rs_in writes complete (semaphore deps, not emission order).
            for k in range(2):
                if not rs_done[k]:
                    emit_rs(k)

            # ---------------- post-collective: add residual, write y ----------------
            # Ordering-only deps keep these AFTER all loop compute in each
            # engine's (strictly in-order) queue — otherwise the scheduler can
            # interleave them mid-loop and the vector/sync engines block on the
            # collective, stalling the whole machine while it runs.
            from concourse.tile import add_dep_helper

            for k in range(2):
                for t in range(2):
                    ro = ypool.tile([PDIM, HALF], bf16, tag="ro")
                    i1 = nc.sync.dma_start(out=ro, in_=rs_out[k][bass.ts(t, PDIM), :])
                    yf = ypool.tile([PDIM, HALF], f32, tag="yf")
                    i2 = nc.vector.tensor_add(yf, ro, resid_sb[t][:, bass.ts(k, HALF)])
                    i3 = nc.sync.dma_start(
                        out=y[bass.ts(t, PDIM), bass.ts(k, HALF)], in_=yf
                    )
                    for ii in (i1, i2, i3):
                        add_dep_helper(
                            ii.ins,
                            last_loop_inst.ins,
                            sync=False,
                            reason="keep post-collective epilogue after loop compute",
                        )

            ps_stack2.close()

    nc.compile()
    return nc


def make_in_maps(x, gn_gamma, gn_beta, w_qkv, w_proj, b_proj, HW):
    """Per-core input dicts. Core c = (b = c//4, h = c%4)."""
    import ml_dtypes

    bf16 = ml_dtypes.bfloat16
    OWN = HW // 4
    x2 = np.ascontiguousarray(x.reshape(B, C, HW).astype(np.float32))
    w_qkv = np.asarray(w_qkv, dtype=np.float32)
    w_proj = np.asarray(w_proj, dtype=np.float32)
    indf = np.zeros((2, PDIM, G), dtype=np.float32)
    indb = np.zeros((2, G, PDIM), dtype=np.float32)
    gsz = C // G  # 32 channels per group
    for t in range(2):
        for p in range(PDIM):
            g = (t * PDIM + p) // gsz
            indf[t, p, g] = 1.0 / gsz
            indb[t, g, p] = 1.0
    in_maps = []
    for c in range(NCORES):
        b, h = c // 4, c % 4
        in_maps.append(
            {
                "xb": x2[b],
                "x_own": np.ascontiguousarray(x2[b][:, h * OWN : (h + 1) * OWN]),
                "wqT": np.ascontiguousarray(
                    np.tile(w_qkv[0 * C + h * D : 0 * C + (h + 1) * D, :].T, (1, 2))
                    * (D ** -0.5 * np.log2(np.e))
                ).astype(bf16),
                "wkT": np.ascontiguousarray(
                    np.tile(w_qkv[1 * C + h * D : 1 * C + (h + 1) * D, :].T, (1, 2))
                ).astype(bf16),
                "wvT": np.ascontiguousarray(w_qkv[2 * C + h * D : 2 * C + (h + 1) * D, :].T).astype(bf16),
                "wpTh": np.ascontiguousarray(w_proj[:, h * D : (h + 1) * D].T).astype(bf16),
                "gamma": np.asarray(gn_gamma, dtype=np.float32),
                "beta": np.asarray(gn_beta, dtype=np.float32),
                "bproj": np.asarray(b_proj, dtype=np.float32),
                "indf": indf,
                "indb": indb,
            }
        )
    return in_maps


def assemble_output(results, HW, Himg, Wimg):
    OWN = HW // 4
    y = np.empty((B, C, HW), dtype=np.float32)
    for c in range(NCORES):
        b, h = c // 4, c % 4
        y[b][:, h * OWN : (h + 1) * OWN] = results[c]["y"]
    return y.reshape(B, C, Himg, Wimg)


_NC_CACHE = {}


def kernel(x, gn_gamma, gn_beta, w_qkv, w_proj, b_proj):
    from concourse.bass_utils import run_bass_kernel_spmd

    Himg, Wimg = x.shape[2], x.shape[3]
    HW = Himg * Wimg
    if HW not in _NC_CACHE:
        _NC_CACHE[HW] = build_nc(HW)
    nc = _NC_CACHE[HW]
    in_maps = make_in_maps(x, gn_gamma, gn_beta, w_qkv, w_proj, b_proj, HW)
    res = run_bass_kernel_spmd(nc, in_maps, list(range(NCORES)))
    return assemble_output(res.results, HW, Himg, Wimg)
